# revision 4
# baseline (speedup 1.0000x reference)
"""Trainium2 Bass kernel for nn_AttentionModel (B=4, S=4096, E=2048) on 8 cores.

Sharding: data-parallel over batch B (4) x tensor-parallel over the E output
dim of the Q projection (2). Core c handles batch b=c//2 and scores rows
e in [h*1024, (h+1)*1024) with h=c%2. Each core computes k, v in full for its
batch (duplicated within the pair; avoids collectives), q for its half, then
scores -> softmax -> attn @ v for its half of the output rows.

All GEMMs run on the PE array in float32r (full-rate fp32, ~1e-4 rel err).
Layouts are chosen so every matmul contracts over the partition dim:
  qT,kT [s, e]: stationary = transposed-x column tiles (host provides x^T)
  v     [f, s]: stationary = Wv^T column tiles, moving = x^T rows
  scores[e, f] = qT.T @ kT contracting s; softmax over free dim f
  outT  [s, e] = v.T @ attnT contracting f (host transposes back)
Q/K biases enter via rank-1 (K=1) matmul accumulation; V bias via the
per-partition bias of the activation-copy eviction. The 1/sqrt(E) score scale
is folded into Wq/bq on the host.
"""

import sys

sys.path.insert(0, "/opt/trn_rl_repo")

from contextlib import ExitStack

import numpy as np

import concourse.bass as bass
import concourse.mybir as mybir
import concourse.tile as tile
from concourse import bacc
from concourse.bass_utils import run_bass_kernel_spmd
from concourse.masks import make_identity

f32 = mybir.dt.float32
f32r = mybir.dt.float32r

B, S, E = 4, 4096, 2048
EH = E // 2          # per-core q rows (embed half)
N = 512              # moving free-dim per matmul (one PSUM bank)
SKT = S // 128       # 32 s k-tiles
EKT = E // 128       # 16 e k-tiles
N_CORES = 8


def build_kernel():
    nc = bacc.Bacc("TRN2", debug=False, target_bir_lowering=False)

    xt = nc.dram_tensor("xt", [E, S], f32r, kind="ExternalInput")        # x^T
    wqk = nc.dram_tensor("wqk", [E, E + EH], f32r, kind="ExternalInput")  # [Wk^T | Wq_h^T/sqrt(E)]
    bkq = nc.dram_tensor("bkq", [1, E + EH], f32r, kind="ExternalInput")  # [bk | bq_h/sqrt(E)]
    wv = nc.dram_tensor("wv", [EKT, E, 128], f32r, kind="ExternalInput")  # Wv^T tiled by f
    bv = nc.dram_tensor("bv", [128, EKT], f32, kind="ExternalInput")      # bv packed per f-tile
    ones_d = nc.dram_tensor("ones", [1, 128], f32r, kind="ExternalInput")
    outt = nc.dram_tensor("outt", [S, EH], f32, kind="ExternalOutput")

    with tile.TileContext(nc) as tc, ExitStack() as ctx:
        dram = ctx.enter_context(tc.tile_pool(name="dram", bufs=1, space="DRAM"))
        qt_d = dram.tile([S, EH], f32r)
        kt_d = dram.tile([S, E], f32r)
        v_d = dram.tile([E, S], f32r)
        sc_d = dram.tile([EH, E], f32)

        const = ctx.enter_context(tc.tile_pool(name="const", bufs=1))
        ones_sb = const.tile([1, 128], f32r)
        nc.sync.dma_start(ones_sb[:, :], ones_d[:, :])
        ident = const.tile([128, 128], f32)
        make_identity(nc, ident[:, :])
        bv_sb = const.tile([128, EKT], f32)
        nc.sync.dma_start(bv_sb[:, :], bv[:, :])
        bkq_sb = const.tile([1, E + EH], f32r)
        nc.sync.dma_start(bkq_sb[:, :], bkq[:, :])

        # ---- Phase 1ab: qT [s, e_h] and kT [s, f] in two f-passes ----
        # pass 0: k cols [0:1024) + q cols (wqk cols [0:1024) and [2048:3072))
        # pass 1: k cols [1024:2048) (wqk cols [1024:2048))
        for p1pass in range(2):
            w_cols = (
                [(0, 1024), (E, E + EH)] if p1pass == 0 else [(1024, 2048)]
            )
            w_width = sum(b - a for a, b in w_cols)
            with (
                tc.tile_pool(name=f"p1_w{p1pass}", bufs=1) as p_w,
                tc.tile_pool(name=f"p1_xc{p1pass}", bufs=3) as p_xc,
                tc.tile_pool(name=f"p1_st{p1pass}", bufs=2) as p_st,
                tc.tile_pool(name=f"p1_ps{p1pass}", bufs=2, space="PSUM") as p_ps,
            ):
                w_sb = p_w.tile([128, EKT, w_width], f32r)
                bias_sb = p_w.tile([1, w_width], f32r)
                off = 0
                for a, b_ in w_cols:
                    nc.sync.dma_start(
                        w_sb[:, :, off:off + (b_ - a)],
                        wqk[:, a:b_].rearrange("(kt p) f -> p kt f", p=128),
                    )
                    nc.sync.dma_start(bias_sb[:, off:off + (b_ - a)], bkq[:, a:b_])
                    off += b_ - a
                nchunks = w_width // N
                for st in range(SKT):
                    xtc = p_xc.tile([128, EKT, 128], f32r, tag="xtc")
                    nc.sync.dma_start(
                        xtc[:, :, :],
                        xt[:, st * 128:(st + 1) * 128].rearrange(
                            "(kt p) s -> p kt s", p=128
                        ),
                    )
                    ps = p_ps.tile([128, w_width], f32, tag="ps")
                    for ekt in range(EKT):
                        lhsT = xtc[:, ekt, :]
                        for fc in range(nchunks):
                            nc.tensor.matmul(
                                ps[:, fc * N:(fc + 1) * N],
                                lhsT,
                                w_sb[:, ekt, fc * N:(fc + 1) * N],
                                start=(ekt == 0),
                                stop=False,
                            )
                    for fc in range(nchunks):
                        nc.tensor.matmul(
                            ps[:, fc * N:(fc + 1) * N],
                            ones_sb[:, :],
                            bias_sb[:, fc * N:(fc + 1) * N],
                            start=False,
                            stop=True,
                        )
                    rows = slice(st * 128, (st + 1) * 128)
                    if p1pass == 0:
                        ksb = p_st.tile([128, 1024], f32r, tag="ksb")
                        nc.scalar.copy(ksb[:, :], ps[:, 0:1024])
                        nc.sync.dma_start(kt_d[rows, 0:1024], ksb[:, :])
                        qsb = p_st.tile([128, EH], f32r, tag="qsb")
                        nc.scalar.copy(qsb[:, :], ps[:, 1024:2048])
                        nc.sync.dma_start(qt_d[rows, :], qsb[:, :])
                    else:
                        ksb = p_st.tile([128, 1024], f32r, tag="ksb")
                        nc.scalar.copy(ksb[:, :], ps[:, 0:1024])
                        nc.sync.dma_start(kt_d[rows, 1024:2048], ksb[:, :])

        # ---- Phase 1c: v [f, s] ----
        with (
            tc.tile_pool(name="p1c_x", bufs=1) as p_xh,
            tc.tile_pool(name="p1c_w", bufs=3) as p_wv,
            tc.tile_pool(name="p1c_st", bufs=3) as p_vst,
            tc.tile_pool(name="p1c_ps", bufs=2, space="PSUM") as p_psv,
        ):
            for sh in range(2):
                xth = p_xh.tile([128, EKT, S // 2], f32r, tag="xth")
                nc.sync.dma_start(
                    xth[:, :, :],
                    xt[:, sh * (S // 2):(sh + 1) * (S // 2)].rearrange(
                        "(kt p) s -> p kt s", p=128
                    ),
                )
                for ft in range(EKT):
                    wvc = p_wv.tile([128, EKT, 128], f32r, tag="wvc")
                    nc.sync.dma_start(
                        wvc[:, :, :],
                        wv[ft].rearrange("(kt p) f -> p kt f", p=128),
                    )
                    psv = p_psv.tile([128, S // 2], f32, tag="psv")
                    for ekt in range(EKT):
                        for sc in range(4):
                            nc.tensor.matmul(
                                psv[:, sc * N:(sc + 1) * N],
                                wvc[:, ekt, :],
                                xth[:, ekt, sc * N:(sc + 1) * N],
                                start=(ekt == 0),
                                stop=(ekt == EKT - 1),
                            )
                    vsb = p_vst.tile([128, S // 2], f32r, tag="vsb")
                    nc.scalar.activation(
                        vsb[:, :], psv[:, :],
                        mybir.ActivationFunctionType.Identity,
                        bias=bv_sb[:, ft:ft + 1], scale=1.0,
                    )
                    nc.sync.dma_start(
                        v_d[ft * 128:(ft + 1) * 128,
                            sh * (S // 2):(sh + 1) * (S // 2)],
                        vsb[:, :],
                    )

        # ---- Phase 2: scores [e_h, f] = qT.T @ kT ----
        with (
            tc.tile_pool(name="p2_k", bufs=1) as p_kh,
            tc.tile_pool(name="p2_q", bufs=2) as p_qc,
            tc.tile_pool(name="p2_st", bufs=3) as p_sst,
            tc.tile_pool(name="p2_ps", bufs=2, space="PSUM") as p_ps2,
        ):
            for fh in range(2):
                kth = p_kh.tile([128, SKT, E // 2], f32r, tag="kth")
                nc.sync.dma_start(
                    kth[:, :, :],
                    kt_d[:, fh * (E // 2):(fh + 1) * (E // 2)].rearrange(
                        "(kt p) f -> p kt f", p=128
                    ),
                )
                for et in range(EH // 128):
                    qtc = p_qc.tile([128, SKT, 128], f32r, tag="qtc")
                    nc.sync.dma_start(
                        qtc[:, :, :],
                        qt_d[:, et * 128:(et + 1) * 128].rearrange(
                            "(kt p) e -> p kt e", p=128
                        ),
                    )
                    ps2 = p_ps2.tile([128, E // 2], f32, tag="ps2")
                    for skt in range(SKT):
                        for fc in range(2):
                            nc.tensor.matmul(
                                ps2[:, fc * N:(fc + 1) * N],
                                qtc[:, skt, :],
                                kth[:, skt, fc * N:(fc + 1) * N],
                                start=(skt == 0),
                                stop=(skt == SKT - 1),
                            )
                    ssb = p_sst.tile([128, E // 2], f32, tag="ssb")
                    nc.scalar.copy(ssb[:, :], ps2[:, :])
                    nc.sync.dma_start(
                        sc_d[et * 128:(et + 1) * 128,
                             fh * (E // 2):(fh + 1) * (E // 2)],
                        ssb[:, :],
                    )

        # ---- Phase 3 + 4: softmax, attn^T, outT = v.T @ attnT ----
        with (
            tc.tile_pool(name="p3_at", bufs=1) as p_at,
            tc.tile_pool(name="p3_sm", bufs=2) as p_sm,
            tc.tile_pool(name="p3_ps", bufs=2, space="PSUM") as p_pst,
        ):
            attnT = p_at.tile([128, EKT, EH], f32r)
            for et in range(EH // 128):
                scs = p_sm.tile([128, E], f32, tag="scs")
                nc.sync.dma_start(scs[:, :], sc_d[et * 128:(et + 1) * 128, :])
                negmax = p_sm.tile([128, 1], f32, tag="negmax")
                nc.vector.tensor_reduce(
                    out=negmax[:, :], in_=scs[:, :], op=mybir.AluOpType.max,
                    axis=mybir.AxisListType.X, negate=True,
                )
                attn = p_sm.tile([128, E], f32, tag="attn")
                sums = p_sm.tile([128, 1], f32, tag="sums")
                nc.scalar.activation(
                    attn[:, :], scs[:, :], mybir.ActivationFunctionType.Exp,
                    bias=negmax[:, 0:1], scale=1.0, accum_out=sums[:, 0:1],
                )
                rsum = p_sm.tile([128, 1], f32, tag="rsum")
                nc.vector.reciprocal(rsum[:, :], sums[:, :])
                attn2 = p_sm.tile([128, E], f32, tag="attn2")
                nc.vector.tensor_scalar_mul(attn2[:, :], attn[:, :], rsum[:, 0:1])
                for half in range(2):
                    pst = p_pst.tile([128, 1024], f32, tag="pst")
                    for c in range(8):
                        fkt = half * 8 + c
                        nc.tensor.transpose(
                            pst[:, c * 128:(c + 1) * 128],
                            attn2[:, fkt * 128:(fkt + 1) * 128],
                            ident[:, :],
                        )
                    nc.vector.tensor_copy(
                        attnT[:, half * 8:(half + 1) * 8,
                              et * 128:(et + 1) * 128],
                        pst[:, :].rearrange("p (c f) -> p c f", f=128),
                    )

            with (
                tc.tile_pool(name="p4_v", bufs=3) as p_vc,
                tc.tile_pool(name="p4_st", bufs=3) as p_ost,
                tc.tile_pool(name="p4_ps", bufs=2, space="PSUM") as p_ps4,
            ):
                for st in range(SKT):
                    vc = p_vc.tile([128, EKT, 128], f32r, tag="vc")
                    nc.sync.dma_start(
                        vc[:, :, :],
                        v_d[:, st * 128:(st + 1) * 128].rearrange(
                            "(kt p) s -> p kt s", p=128
                        ),
                    )
                    ps4 = p_ps4.tile([128, EH], f32, tag="ps4")
                    for fkt in range(EKT):
                        for ec in range(2):
                            nc.tensor.matmul(
                                ps4[:, ec * N:(ec + 1) * N],
                                vc[:, fkt, :],
                                attnT[:, fkt, ec * N:(ec + 1) * N],
                                start=(fkt == 0),
                                stop=(fkt == EKT - 1),
                            )
                    osb = p_ost.tile([128, EH], f32, tag="osb")
                    nc.scalar.copy(osb[:, :], ps4[:, :])
                    nc.sync.dma_start(
                        outt[st * 128:(st + 1) * 128, :], osb[:, :]
                    )

    nc.compile()
    return nc


_NC_CACHE = {}


def _get_nc():
    if "nc" not in _NC_CACHE:
        _NC_CACHE["nc"] = build_kernel()
    return _NC_CACHE["nc"]


def make_in_maps(x, Wq, bq, Wk, bk, Wv, bv):
    sc = np.float32(1.0 / np.sqrt(E))
    in_maps = []
    wk_t = np.ascontiguousarray(Wk.T)                       # [E, E]
    wv_t = np.ascontiguousarray(Wv.T)                       # [E, E]
    wv_tiled = np.ascontiguousarray(
        wv_t.reshape(E, EKT, 128).transpose(1, 0, 2)        # [EKT, E, 128]
    )
    bv_packed = np.ascontiguousarray(bv.reshape(EKT, 128).T)  # [128, EKT]
    for c in range(N_CORES):
        b, h = c // 2, c % 2
        xt = np.ascontiguousarray(x[b].T)                   # [E, S]
        wq_h = Wq[h * EH:(h + 1) * EH, :] * sc              # [EH, E]
        wqk = np.ascontiguousarray(
            np.concatenate([wk_t, wq_h.T], axis=1)          # [E, E+EH]
        )
        bkq = np.concatenate([bk, bq[h * EH:(h + 1) * EH] * sc])[None, :]
        in_maps.append({
            "xt": xt,
            "wqk": wqk,
            "bkq": np.ascontiguousarray(bkq.astype(np.float32)),
            "wv": wv_tiled,
            "bv": bv_packed,
            "ones": np.ones((1, 128), np.float32),
        })
    return in_maps


def run(in_maps, trace=False, **kwargs):
    nc = _get_nc()
    return run_bass_kernel_spmd(
        nc, in_maps, core_ids=list(range(N_CORES)), trace=trace, **kwargs
    )


def kernel(x, Wq, bq, Wk, bk, Wv, bv):
    x = np.asarray(x, dtype=np.float32)
    in_maps = make_in_maps(
        x,
        np.asarray(Wq, np.float32), np.asarray(bq, np.float32),
        np.asarray(Wk, np.float32), np.asarray(bk, np.float32),
        np.asarray(Wv, np.float32), np.asarray(bv, np.float32),
    )
    res = run(in_maps, trace=False)
    out = np.empty((B, E, S), dtype=np.float32)
    for c in range(N_CORES):
        b, h = c // 2, c % 2
        out[b, h * EH:(h + 1) * EH, :] = res.results[c]["outt"].T
    return out


# revision 5
# speedup vs baseline: 1.0508x; 1.0508x over previous
"""Trainium2 Bass kernel for nn_AttentionModel (B=4, S=4096, E=2048) on 8 cores.

Sharding: data-parallel over batch B (4) x tensor-parallel over the E output
dim of the Q projection (2). Core c handles batch b=c//2 and scores rows
e in [h*1024, (h+1)*1024) with h=c%2. Each core computes k, v in full for its
batch (duplicated within the pair; avoids collectives), q for its half, then
scores -> softmax -> attn @ v for its half of the output rows.

All GEMMs run on the PE array in float32r (full-rate fp32, ~1e-4 rel err).
Layouts are chosen so every matmul contracts over the partition dim:
  qT,kT [s, e]: stationary = transposed-x column tiles (host provides x^T)
  v     [f, s]: stationary = Wv^T column tiles, moving = x^T rows
  scores[e, f] = qT.T @ kT contracting s; softmax over free dim f
  outT  [s, e] = v.T @ attnT contracting f (host transposes back)
Q/K biases enter via rank-1 (K=1) matmul accumulation; V bias via the
per-partition bias of the activation-copy eviction. The 1/sqrt(E) score scale
is folded into Wq/bq on the host.
"""

import sys

sys.path.insert(0, "/opt/trn_rl_repo")

from contextlib import ExitStack

import numpy as np

import concourse.bass as bass
import concourse.mybir as mybir
import concourse.tile as tile
from concourse import bacc
from concourse.bass_utils import run_bass_kernel_spmd
from concourse.masks import make_identity

f32 = mybir.dt.float32
f32r = mybir.dt.float32r

B, S, E = 4, 4096, 2048
EH = E // 2          # per-core q rows (embed half)
N = 512              # moving free-dim per matmul (one PSUM bank)
SKT = S // 128       # 32 s k-tiles
EKT = E // 128       # 16 e k-tiles
N_CORES = 8


def build_kernel():
    nc = bacc.Bacc("TRN2", debug=False, target_bir_lowering=False)

    xt = nc.dram_tensor("xt", [E, S], f32r, kind="ExternalInput")        # x^T
    wqk = nc.dram_tensor("wqk", [E, E + EH], f32r, kind="ExternalInput")  # [Wk^T | Wq_h^T/sqrt(E)]
    bkq = nc.dram_tensor("bkq", [1, E + EH], f32r, kind="ExternalInput")  # [bk | bq_h/sqrt(E)]
    wv = nc.dram_tensor("wv", [EKT, E, 128], f32r, kind="ExternalInput")  # Wv^T tiled by f
    bv = nc.dram_tensor("bv", [128, EKT], f32, kind="ExternalInput")      # bv packed per f-tile
    ones_d = nc.dram_tensor("ones", [1, 128], f32r, kind="ExternalInput")
    outt = nc.dram_tensor("outt", [S, EH], f32, kind="ExternalOutput")

    with tile.TileContext(nc) as tc, ExitStack() as ctx:
        dram = ctx.enter_context(tc.tile_pool(name="dram", bufs=1, space="DRAM"))
        qt_d = dram.tile([S, EH], f32r)
        kt_d = dram.tile([S, E], f32r)
        v_d = dram.tile([E, S], f32r)
        sc_d = dram.tile([EH, E], f32)

        const = ctx.enter_context(tc.tile_pool(name="const", bufs=1))
        ones_sb = const.tile([1, 128], f32r)
        nc.sync.dma_start(ones_sb[:, :], ones_d[:, :])
        ident = const.tile([128, 128], f32)
        make_identity(nc, ident[:, :])
        bv_sb = const.tile([128, EKT], f32)
        nc.sync.dma_start(bv_sb[:, :], bv[:, :])
        bkq_sb = const.tile([1, E + EH], f32r)
        nc.sync.dma_start(bkq_sb[:, :], bkq[:, :])

        # ---- Phase 1ab: qT [s, e_h] and kT [s, f] in two f-passes ----
        # pass 0: k cols [0:1024) + q cols (wqk cols [0:1024) and [2048:3072))
        # pass 1: k cols [1024:2048) (wqk cols [1024:2048))
        for p1pass in range(2):
            w_cols = (
                [(0, 1024), (E, E + EH)] if p1pass == 0 else [(1024, 2048)]
            )
            w_width = sum(b - a for a, b in w_cols)
            with (
                tc.tile_pool(name=f"p1_w{p1pass}", bufs=1) as p_w,
                tc.tile_pool(name=f"p1_xc{p1pass}", bufs=3) as p_xc,
                tc.tile_pool(name=f"p1_st{p1pass}", bufs=2) as p_st,
                tc.tile_pool(name=f"p1_ps{p1pass}", bufs=2, space="PSUM") as p_ps,
            ):
                w_sb = p_w.tile([128, EKT, w_width], f32r)
                bias_sb = p_w.tile([1, w_width], f32r)
                off = 0
                for a, b_ in w_cols:
                    for ekt in range(EKT):
                        nc.sync.dma_start(
                            w_sb[:, ekt, off:off + (b_ - a)],
                            wqk[ekt * 128:(ekt + 1) * 128, a:b_],
                        )
                    nc.sync.dma_start(bias_sb[:, off:off + (b_ - a)], bkq[:, a:b_])
                    off += b_ - a
                nchunks = w_width // N
                for st in range(SKT):
                    xtc = p_xc.tile([128, EKT, 128], f32r, tag="xtc")
                    nc.sync.dma_start(
                        xtc[:, :, :],
                        xt[:, st * 128:(st + 1) * 128].rearrange(
                            "(kt p) s -> p kt s", p=128
                        ),
                    )
                    ps = p_ps.tile([128, w_width], f32, tag="ps")
                    for ekt in range(EKT):
                        lhsT = xtc[:, ekt, :]
                        for fc in range(nchunks):
                            nc.tensor.matmul(
                                ps[:, fc * N:(fc + 1) * N],
                                lhsT,
                                w_sb[:, ekt, fc * N:(fc + 1) * N],
                                start=(ekt == 0),
                                stop=False,
                            )
                    for fc in range(nchunks):
                        nc.tensor.matmul(
                            ps[:, fc * N:(fc + 1) * N],
                            ones_sb[:, :],
                            bias_sb[:, fc * N:(fc + 1) * N],
                            start=False,
                            stop=True,
                        )
                    rows = slice(st * 128, (st + 1) * 128)
                    if p1pass == 0:
                        ksb = p_st.tile([128, 1024], f32r, tag="ksb")
                        nc.scalar.copy(ksb[:, :], ps[:, 0:1024])
                        nc.sync.dma_start(kt_d[rows, 0:1024], ksb[:, :])
                        qsb = p_st.tile([128, EH], f32r, tag="qsb")
                        nc.scalar.copy(qsb[:, :], ps[:, 1024:2048])
                        nc.sync.dma_start(qt_d[rows, :], qsb[:, :])
                    else:
                        ksb = p_st.tile([128, 1024], f32r, tag="ksb")
                        nc.scalar.copy(ksb[:, :], ps[:, 0:1024])
                        nc.sync.dma_start(kt_d[rows, 1024:2048], ksb[:, :])

        # ---- Phase 1c: v [f, s] ----
        with (
            tc.tile_pool(name="p1c_x", bufs=1) as p_xh,
            tc.tile_pool(name="p1c_w", bufs=3) as p_wv,
            tc.tile_pool(name="p1c_st", bufs=3) as p_vst,
            tc.tile_pool(name="p1c_ps", bufs=2, space="PSUM") as p_psv,
        ):
            for sh in range(2):
                xth = p_xh.tile([128, EKT, S // 2], f32r, tag="xth")
                for ekt in range(EKT):
                    nc.sync.dma_start(
                        xth[:, ekt, :],
                        xt[ekt * 128:(ekt + 1) * 128,
                           sh * (S // 2):(sh + 1) * (S // 2)],
                    )
                for ft in range(EKT):
                    wvc = p_wv.tile([128, EKT, 128], f32r, tag="wvc")
                    nc.sync.dma_start(
                        wvc[:, :, :],
                        wv[ft].rearrange("(kt p) f -> p kt f", p=128),
                    )
                    psv = p_psv.tile([128, S // 2], f32, tag="psv")
                    for ekt in range(EKT):
                        for sc in range(4):
                            nc.tensor.matmul(
                                psv[:, sc * N:(sc + 1) * N],
                                wvc[:, ekt, :],
                                xth[:, ekt, sc * N:(sc + 1) * N],
                                start=(ekt == 0),
                                stop=(ekt == EKT - 1),
                            )
                    vsb = p_vst.tile([128, S // 2], f32r, tag="vsb")
                    nc.scalar.activation(
                        vsb[:, :], psv[:, :],
                        mybir.ActivationFunctionType.Identity,
                        bias=bv_sb[:, ft:ft + 1], scale=1.0,
                    )
                    nc.sync.dma_start(
                        v_d[ft * 128:(ft + 1) * 128,
                            sh * (S // 2):(sh + 1) * (S // 2)],
                        vsb[:, :],
                    )

        # ---- Phase 2: scores [e_h, f] = qT.T @ kT ----
        with (
            tc.tile_pool(name="p2_k", bufs=1) as p_kh,
            tc.tile_pool(name="p2_q", bufs=2) as p_qc,
            tc.tile_pool(name="p2_st", bufs=3) as p_sst,
            tc.tile_pool(name="p2_ps", bufs=2, space="PSUM") as p_ps2,
        ):
            for fh in range(2):
                kth = p_kh.tile([128, SKT, E // 2], f32r, tag="kth")
                for skt in range(SKT):
                    nc.sync.dma_start(
                        kth[:, skt, :],
                        kt_d[skt * 128:(skt + 1) * 128,
                             fh * (E // 2):(fh + 1) * (E // 2)],
                    )
                for et in range(EH // 128):
                    qtc = p_qc.tile([128, SKT, 128], f32r, tag="qtc")
                    nc.sync.dma_start(
                        qtc[:, :, :],
                        qt_d[:, et * 128:(et + 1) * 128].rearrange(
                            "(kt p) e -> p kt e", p=128
                        ),
                    )
                    ps2 = p_ps2.tile([128, E // 2], f32, tag="ps2")
                    for skt in range(SKT):
                        for fc in range(2):
                            nc.tensor.matmul(
                                ps2[:, fc * N:(fc + 1) * N],
                                qtc[:, skt, :],
                                kth[:, skt, fc * N:(fc + 1) * N],
                                start=(skt == 0),
                                stop=(skt == SKT - 1),
                            )
                    ssb = p_sst.tile([128, E // 2], f32, tag="ssb")
                    nc.scalar.copy(ssb[:, :], ps2[:, :])
                    nc.sync.dma_start(
                        sc_d[et * 128:(et + 1) * 128,
                             fh * (E // 2):(fh + 1) * (E // 2)],
                        ssb[:, :],
                    )

        # ---- Phase 3 + 4: softmax, attn^T, outT = v.T @ attnT ----
        with (
            tc.tile_pool(name="p3_at", bufs=1) as p_at,
            tc.tile_pool(name="p3_sm", bufs=2) as p_sm,
            tc.tile_pool(name="p3_ps", bufs=2, space="PSUM") as p_pst,
        ):
            attnT = p_at.tile([128, EKT, EH], f32r)
            for et in range(EH // 128):
                scs = p_sm.tile([128, E], f32, tag="scs")
                nc.sync.dma_start(scs[:, :], sc_d[et * 128:(et + 1) * 128, :])
                negmax = p_sm.tile([128, 1], f32, tag="negmax")
                nc.vector.tensor_reduce(
                    out=negmax[:, :], in_=scs[:, :], op=mybir.AluOpType.max,
                    axis=mybir.AxisListType.X, negate=True,
                )
                attn = p_sm.tile([128, E], f32, tag="attn")
                sums = p_sm.tile([128, 1], f32, tag="sums")
                nc.scalar.activation(
                    attn[:, :], scs[:, :], mybir.ActivationFunctionType.Exp,
                    bias=negmax[:, 0:1], scale=1.0, accum_out=sums[:, 0:1],
                )
                rsum = p_sm.tile([128, 1], f32, tag="rsum")
                nc.vector.reciprocal(rsum[:, :], sums[:, :])
                attn2 = p_sm.tile([128, E], f32, tag="attn2")
                nc.vector.tensor_scalar_mul(attn2[:, :], attn[:, :], rsum[:, 0:1])
                for half in range(2):
                    pst = p_pst.tile([128, 1024], f32, tag="pst")
                    for c in range(8):
                        fkt = half * 8 + c
                        nc.tensor.transpose(
                            pst[:, c * 128:(c + 1) * 128],
                            attn2[:, fkt * 128:(fkt + 1) * 128],
                            ident[:, :],
                        )
                    nc.vector.tensor_copy(
                        attnT[:, half * 8:(half + 1) * 8,
                              et * 128:(et + 1) * 128],
                        pst[:, :].rearrange("p (c f) -> p c f", f=128),
                    )

            with (
                tc.tile_pool(name="p4_v", bufs=3) as p_vc,
                tc.tile_pool(name="p4_st", bufs=3) as p_ost,
                tc.tile_pool(name="p4_ps", bufs=2, space="PSUM") as p_ps4,
            ):
                for st in range(SKT):
                    vc = p_vc.tile([128, EKT, 128], f32r, tag="vc")
                    nc.sync.dma_start(
                        vc[:, :, :],
                        v_d[:, st * 128:(st + 1) * 128].rearrange(
                            "(kt p) s -> p kt s", p=128
                        ),
                    )
                    ps4 = p_ps4.tile([128, EH], f32, tag="ps4")
                    for fkt in range(EKT):
                        for ec in range(2):
                            nc.tensor.matmul(
                                ps4[:, ec * N:(ec + 1) * N],
                                vc[:, fkt, :],
                                attnT[:, fkt, ec * N:(ec + 1) * N],
                                start=(fkt == 0),
                                stop=(fkt == EKT - 1),
                            )
                    osb = p_ost.tile([128, EH], f32, tag="osb")
                    nc.scalar.copy(osb[:, :], ps4[:, :])
                    nc.sync.dma_start(
                        outt[st * 128:(st + 1) * 128, :], osb[:, :]
                    )

    nc.compile()
    return nc


_NC_CACHE = {}


def _get_nc():
    if "nc" not in _NC_CACHE:
        _NC_CACHE["nc"] = build_kernel()
    return _NC_CACHE["nc"]


def make_in_maps(x, Wq, bq, Wk, bk, Wv, bv):
    sc = np.float32(1.0 / np.sqrt(E))
    in_maps = []
    wk_t = np.ascontiguousarray(Wk.T)                       # [E, E]
    wv_t = np.ascontiguousarray(Wv.T)                       # [E, E]
    wv_tiled = np.ascontiguousarray(
        wv_t.reshape(E, EKT, 128).transpose(1, 0, 2)        # [EKT, E, 128]
    )
    bv_packed = np.ascontiguousarray(bv.reshape(EKT, 128).T)  # [128, EKT]
    for c in range(N_CORES):
        b, h = c // 2, c % 2
        xt = np.ascontiguousarray(x[b].T)                   # [E, S]
        wq_h = Wq[h * EH:(h + 1) * EH, :] * sc              # [EH, E]
        wqk = np.ascontiguousarray(
            np.concatenate([wk_t, wq_h.T], axis=1)          # [E, E+EH]
        )
        bkq = np.concatenate([bk, bq[h * EH:(h + 1) * EH] * sc])[None, :]
        in_maps.append({
            "xt": xt,
            "wqk": wqk,
            "bkq": np.ascontiguousarray(bkq.astype(np.float32)),
            "wv": wv_tiled,
            "bv": bv_packed,
            "ones": np.ones((1, 128), np.float32),
        })
    return in_maps


def run(in_maps, trace=False, **kwargs):
    nc = _get_nc()
    return run_bass_kernel_spmd(
        nc, in_maps, core_ids=list(range(N_CORES)), trace=trace, **kwargs
    )


def kernel(x, Wq, bq, Wk, bk, Wv, bv):
    x = np.asarray(x, dtype=np.float32)
    in_maps = make_in_maps(
        x,
        np.asarray(Wq, np.float32), np.asarray(bq, np.float32),
        np.asarray(Wk, np.float32), np.asarray(bk, np.float32),
        np.asarray(Wv, np.float32), np.asarray(bv, np.float32),
    )
    res = run(in_maps, trace=False)
    out = np.empty((B, E, S), dtype=np.float32)
    for c in range(N_CORES):
        b, h = c // 2, c % 2
        out[b, h * EH:(h + 1) * EH, :] = res.results[c]["outt"].T
    return out
